# revision 2
# baseline (speedup 1.0000x reference)
"""Gridnet kernel for 8x Trainium2 NeuronCores — self-contained.

Computes the blockwise-normalized 27-neighbor gridnet step (8 inner
iterations, block_size 8) for x:[4,128,128,128] with per-cell weights
w:[27,128,128,128], bias, residual_scale.

Distribution: the M axis is sharded over the 8 cores (16 rows each = 512
spatial 8^3 blocks per core).  The 1-cell halo of every block is frozen at
the initial activations, so blocks are independent for the whole step: each
core runs all 8 iterations locally with zero collectives.

Per core the 512 blocks are processed as 4 chunks of 128 (partition <->
block).  Engines: DVE does the 27 per-cell fp16 window products (2x packed
mode; two 1-element-shifted activation mirrors keep every window read
4B-aligned) plus Newton-sqrt statistics; the Tensor engine sums the 27
product planes + the normalization-correction plane into PSUM with fp32
identity-matmul accumulation; the Scalar engine applies
silu(istd*(S+P0)) with a per-partition scale and maintains the fp16
mirrors (interior sums ride along for free via accum_out).  The fp32
master copy of the activations carries the residual stream.

Falls back to a pure-numpy implementation if the device path fails.
"""
import dataclasses
import numpy as np

BS = 8
PAD = 10
PADC = 1000
EPS = 1e-5
OFFSETS = [(i, j, k) for i in range(3) for j in range(3) for k in range(3)]

_CACHE = {}


# ------------------------------------------------------------- host layout

def _prep_core(weight, bias, rscale, x, core):
    w = weight.astype(np.float32)
    b = bias.astype(np.float32)
    rs = rscale.astype(np.float32)
    x = x.astype(np.float32)
    B = x.shape[0]
    m0 = core * 16

    ws = w[:, m0:m0 + 16]
    ws = ws.reshape(27, 2, 8, 2, 8, 8, 16, 8)
    ws = ws.transpose(1, 3, 4, 6, 0, 2, 5, 7)           # bm,bnh,bnl,bk,o,u,v,w
    wq = ws.reshape(4, 128, 27, 512)

    def blk_param(p3):
        y = p3.reshape(2, 8, 2, 8, 8, 16, 8)
        y = y.transpose(0, 2, 3, 5, 1, 4, 6)
        return y.reshape(4, 128, 512)

    bq = blk_param(b[m0:m0 + 16])
    rsq = blk_param(rs[m0:m0 + 16])
    swq = wq.sum(axis=2)

    xp = np.pad(x, ((0, 0), (1, 1), (1, 1), (1, 1)))[:, m0:m0 + 18]
    sw_v = np.lib.stride_tricks.sliding_window_view(xp, (PAD, PAD, PAD), axis=(1, 2, 3))
    blocks = sw_v[:, ::BS, ::BS, ::BS]
    blocks = blocks.reshape(B, 2, 2, 8, 16, PAD, PAD, PAD)
    blocks = blocks.transpose(1, 2, 3, 4, 0, 5, 6, 7)
    acts0 = blocks.reshape(4, 128, B, PADC)

    xs = x[:, m0:m0 + 16]
    xs = xs.reshape(B, 2, 8, 2, 8, 8, 16, 8)
    xs = xs.transpose(1, 3, 4, 6, 0, 2, 5, 7)
    master0 = xs.reshape(4, 128, B, 512)

    pad_sum = acts0.sum(-1)
    pad_sumsq = (acts0 * acts0).sum(-1)
    halo_sum = pad_sum - master0.sum(-1)
    halo_sumsq = pad_sumsq - (master0 * master0).sum(-1)
    pad0 = np.stack([pad_sum, pad_sumsq], axis=-1).reshape(4, 128, 2 * B)
    halo = np.stack([halo_sum, halo_sumsq], axis=-1).reshape(4, 128, 2 * B)
    st_in = np.concatenate([pad0, halo], axis=-1)

    return {
        "wq": np.ascontiguousarray(wq.reshape(4, 128, 27 * 512)).astype(np.float16),
        "swq": np.ascontiguousarray(swq).astype(np.float16),
        "bq": np.ascontiguousarray(bq).astype(np.float16),
        "rsq": np.ascontiguousarray(rsq).astype(np.float16),
        "acts0": np.ascontiguousarray(acts0.reshape(4, 128, B * PADC)).astype(np.float16),
        "master0": np.ascontiguousarray(master0.reshape(4, 128, B * 512)).astype(np.float32),
        "st_in": np.ascontiguousarray(st_in).astype(np.float32),
        "ident": np.eye(128, dtype=np.float16),
    }


def _unshard(outs, B=4):
    full = np.empty((B, 128, 128, 128), np.float32)
    for core, o in enumerate(outs):
        m0 = core * 16
        y = o.reshape(4, 128, B, 512)
        y = y.reshape(2, 2, 8, 16, B, 8, 8, 8)
        y = y.transpose(4, 0, 5, 1, 2, 6, 3, 7)
        full[:, m0:m0 + 16] = y.reshape(B, 16, 128, 128)
    return full


# ------------------------------------------------------------- bass build

def _split_multi_waits(nc, mybir):
    ctr = 0
    for f in nc.m.functions:
        for bb in f.blocks:
            insts = list(bb.instructions)
            if not any(i.sync_info and i.sync_info.on_wait and len(i.sync_info.on_wait) > 1
                       for i in insts):
                continue
            out = []
            for ins in insts:
                si = ins.sync_info
                waits = list(si.on_wait) if (si and si.on_wait) else []
                if len(waits) > 1:
                    for wv in waits[:-1]:
                        ctr += 1
                        nop = mybir.InstNoOp(name=f"I-wsplit-{ctr}", engine=ins.engine,
                                             ins=[], outs=[])
                        nop.sync_info = mybir.SyncInfo(on_wait=[wv], on_update=[])
                        out.append(nop)
                    ins.sync_info = mybir.SyncInfo(
                        on_wait=[waits[-1]],
                        on_update=list(si.on_update) if si.on_update else [])
                out.append(ins)
            bb.instructions = out
    return ctr


def _sub(ap, dims, extra_off):
    return dataclasses.replace(ap, ap=[list(ap.ap[0])] + dims, offset=ap.offset + extra_off)


def _build(nc, NB=4, n_iters=8, newton=5, silu_mode="act",
           skip_bias=False, skip_rs=False, gpsimd_planes=4):
    import concourse.tile as tile
    import concourse.mybir as mybir
    from contextlib import ExitStack

    F16 = mybir.dt.float16
    F32 = mybir.dt.float32
    AF = mybir.ActivationFunctionType
    ALU = mybir.AluOpType
    n_chunks = 4

    wq_d = nc.dram_tensor("wq", [n_chunks, 128, 27 * 512], F16, kind="ExternalInput")
    swq_d = nc.dram_tensor("swq", [n_chunks, 128, 512], F16, kind="ExternalInput")
    bq_d = nc.dram_tensor("bq", [n_chunks, 128, 512], F16, kind="ExternalInput")
    rsq_d = nc.dram_tensor("rsq", [n_chunks, 128, 512], F16, kind="ExternalInput")
    acts0_d = nc.dram_tensor("acts0", [n_chunks, 128, NB * PADC], F16, kind="ExternalInput")
    master0_d = nc.dram_tensor("master0", [n_chunks, 128, NB * 512], F32, kind="ExternalInput")
    st_d = nc.dram_tensor("st_in", [n_chunks, 128, 4 * NB], F32, kind="ExternalInput")
    id_d = nc.dram_tensor("ident", [128, 128], F16, kind="ExternalInput")
    out_d = nc.dram_tensor("outp", [n_chunks, 128, NB * 512], F32, kind="ExternalOutput")

    with tile.TileContext(nc) as tc, ExitStack() as es:
        consts = es.enter_context(tc.tile_pool(name="consts", bufs=1))
        wpool = es.enter_context(tc.tile_pool(name="wpool", bufs=2))
        apool = es.enter_context(tc.tile_pool(name="apool", bufs=2))
        ppool = es.enter_context(tc.tile_pool(name="ppool", bufs=3))
        zpool = es.enter_context(tc.tile_pool(name="zpool", bufs=3))
        spool = es.enter_context(tc.tile_pool(name="spool", bufs=4))
        pspool = es.enter_context(tc.tile_pool(name="pspool", bufs=4, space="PSUM"))

        ident = consts.tile([128, 128], F16)
        nc.sync.dma_start(out=ident, in_=id_d[:, :])

        for c in range(n_chunks):
            wt = wpool.tile([128, 27, 512], F16, tag="wt")
            swt = wpool.tile([128, 512], F16, tag="swt")
            bt = wpool.tile([128, 512], F16, tag="bt")
            rst = wpool.tile([128, 512], F16, tag="rst")
            mirAs = [apool.tile([128, PADC], F16, tag=f"mirA{b}", name=f"mirA{b}")
                     for b in range(NB)]
            mirBs = [apool.tile([128, PADC], F16, tag=f"mirB{b}", name=f"mirB{b}")
                     for b in range(NB)]
            master = apool.tile([128, NB, 512], F32, tag="master")
            stt = apool.tile([128, 4 * NB], F32, tag="stt")

            nc.sync.dma_start(out=wt, in_=wq_d[c].rearrange("p (o c) -> p o c", o=27))
            nc.sync.dma_start(out=swt, in_=swq_d[c])
            nc.sync.dma_start(out=bt, in_=bq_d[c])
            nc.sync.dma_start(out=rst, in_=rsq_d[c])
            for b in range(NB):
                nc.sync.dma_start(out=mirAs[b], in_=acts0_d[c][:, b * PADC:(b + 1) * PADC])
            nc.sync.dma_start(out=master, in_=master0_d[c].rearrange("p (b c) -> p b c", b=NB))
            nc.sync.dma_start(out=stt, in_=st_d[c])

            for b in range(NB):
                nc.scalar.activation(out=mirBs[b][:, 0:PADC - 1],
                                     in_=mirAs[b][:, 1:PADC], func=AF.Copy)

            st_prev = None
            for it in range(n_iters):
                if it == 0:
                    sums = stt[:, 0:2 * NB]
                else:
                    sums_t = spool.tile([128, 2 * NB], F32, tag="sums")
                    nc.vector.tensor_add(sums_t, st_prev[:, :], stt[:, 2 * NB:4 * NB])
                    sums = sums_t[:, :]
                m8 = spool.tile([128, 2 * NB], F32, tag="m8")
                nc.vector.tensor_scalar_mul(m8, sums, 1.0 / PADC)
                m8a = m8[:, :]
                mu = _sub(m8a, [[2, NB]], 0)
                E2 = _sub(m8a, [[2, NB]], 1)
                musq = spool.tile([128, NB], F32, tag="musq")
                nc.vector.tensor_mul(musq, mu, mu)
                a4 = spool.tile([128, NB], F32, tag="a4")
                nc.vector.tensor_sub(a4, E2, musq)
                nc.vector.tensor_scalar(a4, a4, 1.0, EPS, ALU.mult, ALU.add)
                s4 = spool.tile([128, NB], F32, tag="s4")
                nc.vector.tensor_scalar(s4, a4, 0.5, 0.5, ALU.mult, ALU.add)
                for _ in range(newton):
                    rr = spool.tile([128, NB], F32, tag="rr")
                    nc.vector.reciprocal(rr, s4)
                    ar = spool.tile([128, NB], F32, tag="ar")
                    nc.vector.tensor_mul(ar, a4, rr)
                    nc.vector.tensor_add(ar, ar, s4)
                    nc.vector.tensor_scalar_mul(s4, ar, 0.5)
                istd = spool.tile([128, NB], F32, tag="istd")
                nc.vector.reciprocal(istd, s4)
                negmu = spool.tile([128, NB], F32, tag="negmu")
                nc.vector.tensor_scalar_mul(negmu, mu, -1.0)

                prods = {}
                for o, (di, dj, dk) in enumerate(OFFSETS):
                    base = di * 100 + dj * 10 + dk
                    wo = wt[:, o, :]
                    eng = nc.gpsimd if o < gpsimd_planes else nc.vector
                    for b in range(NB):
                        srct, off = (mirAs[b], base) if dk != 1 else (mirBs[b], base - 1)
                        win = _sub(srct[:, :], [[100, 8], [10, 8], [1, 8]], off)
                        wsrc = _sub(wo, [[64, 8], [8, 8], [1, 8]], 0)
                        pt = ppool.tile([128, 512], F16, tag=f"prod{(o * NB + b) % 6}",
                                        name=f"pt{o}_{b}")
                        pdst = _sub(pt[:, :], [[64, 8], [8, 8], [1, 8]], 0)
                        eng.tensor_tensor(pdst, wsrc, win, ALU.mult)
                        prods[(o, b)] = pt[:, :]

                for b in range(NB):
                    t1 = zpool.tile([128, 512], F16, tag="t1")
                    nc.vector.tensor_scalar_mul(t1, swt, negmu[:, b:b + 1])
                    if skip_bias:
                        p0 = t1
                    else:
                        t2 = zpool.tile([128, 512], F16, tag="t2")
                        nc.scalar.activation(out=t2, in_=bt, func=AF.Copy,
                                             scale=s4[:, b:b + 1])
                        p0 = zpool.tile([128, 512], F16, tag="p0")
                        nc.vector.tensor_add(p0, t1, t2)

                    ps = pspool.tile([128, 512], F32, tag=f"ps{b % 2}")
                    nc.tensor.matmul(ps, ident, p0, start=True, stop=False)
                    for o in range(27):
                        nc.tensor.matmul(ps, ident, prods[(o, b)],
                                         start=False, stop=(o == 26))
                    if silu_mode == "act":
                        z = zpool.tile([128, 512], F16, tag="z")
                        nc.scalar.activation(out=z, in_=ps, func=AF.Silu,
                                             scale=istd[:, b:b + 1])
                        if skip_rs:
                            rz = z
                        else:
                            rz = zpool.tile([128, 512], F16, tag="rz")
                            nc.vector.tensor_mul(rz, rst, z)
                    else:
                        sg = zpool.tile([128, 512], F16, tag="z")
                        nc.scalar.activation(out=sg, in_=ps, func=AF.Sigmoid,
                                             scale=istd[:, b:b + 1])
                        t = zpool.tile([128, 512], F16, tag="tz")
                        nc.vector.tensor_mul(t, ps, sg)
                        nc.vector.tensor_scalar_mul(t, t, istd[:, b:b + 1])
                        rz = zpool.tile([128, 512], F16, tag="rz")
                        nc.vector.tensor_mul(rz, rst, t)
                    nc.vector.tensor_add(master[:, b, :], master[:, b, :], rz)

                if it + 1 < n_iters:
                    st_cur = spool.tile([128, 2 * NB], F32, tag="stc")
                    for b in range(NB):
                        intA = _sub(mirAs[b][:, :], [[100, 8], [10, 8], [1, 8]], 111)
                        intB = _sub(mirBs[b][:, :], [[100, 8], [10, 8], [1, 8]], 110)
                        nc.scalar.activation(out=intA, in_=master[:, b, :], func=AF.Copy,
                                             accum_out=st_cur[:, 2 * b:2 * b + 1])
                        nc.scalar.activation(out=intB, in_=master[:, b, :], func=AF.Copy)
                        sqd = zpool.tile([128, 512], F32, tag="sqd")
                        nc.scalar.activation(out=sqd, in_=master[:, b, :], func=AF.Square,
                                             accum_out=st_cur[:, 2 * b + 1:2 * b + 2])
                    st_prev = st_cur

            nc.sync.dma_start(out=out_d[c],
                              in_=master[:, :, :].rearrange("p b c -> p (b c)"))
    return nc


def _get_runner(silu_mode="act", skip_bias=False, skip_rs=False):
    key = ("runner", silu_mode, skip_bias, skip_rs)
    if key in _CACHE:
        return _CACHE[key]
    import concourse.bass as bass
    import concourse.mybir as mybir
    import jax
    from jax.sharding import Mesh, PartitionSpec
    from jax.experimental.shard_map import shard_map
    from concourse import bass2jax
    from concourse.bass2jax import _bass_exec_p, install_neuronx_cc_hook

    nc = bass.Bass()
    _build(nc, silu_mode=silu_mode, skip_bias=skip_bias, skip_rs=skip_rs)
    _split_multi_waits(nc, mybir)

    install_neuronx_cc_hook()
    partition_name = nc.partition_id_tensor.name if nc.partition_id_tensor else None
    in_names, out_names, out_avals, zero_outs = [], [], [], []
    for alloc in nc.m.functions[0].allocations:
        if not isinstance(alloc, mybir.MemoryLocationSet):
            continue
        name = alloc.memorylocations[0].name
        if alloc.kind == "ExternalInput":
            if name != partition_name:
                in_names.append(name)
        elif alloc.kind == "ExternalOutput":
            out_names.append(name)
            shape = tuple(alloc.tensor_shape)
            dtype = mybir.dt.np(alloc.dtype)
            out_avals.append(jax.core.ShapedArray(shape, dtype))
            zero_outs.append(np.zeros(shape, dtype))
    all_in_names = in_names + out_names
    if partition_name is not None:
        all_in_names.append(partition_name)

    def _body(*args):
        operands = list(args)
        if partition_name is not None:
            operands.append(bass2jax.partition_id_tensor())
        outs = _bass_exec_p.bind(
            *operands,
            out_avals=tuple(out_avals),
            in_names=tuple(all_in_names),
            out_names=tuple(out_names),
            lowering_input_output_aliases=(),
            sim_require_finite=True,
            sim_require_nnan=True,
            nc=nc,
        )
        return tuple(outs)

    devices = jax.devices()[:8]
    mesh = Mesh(np.asarray(devices), ("core",))
    n_all = len(in_names) + len(out_names)
    fn = jax.jit(
        shard_map(_body, mesh=mesh,
                  in_specs=(PartitionSpec("core"),) * n_all,
                  out_specs=(PartitionSpec("core"),) * len(out_names),
                  check_rep=False),
        keep_unused=True)

    runner = {"fn": fn, "in_names": in_names, "out_names": out_names,
              "out_avals": out_avals, "zero_outs": zero_outs}
    _CACHE[key] = runner
    return runner


def _run_device(weight, bias, rscale, x):
    r = _get_runner(skip_bias=bool(np.all(bias == 0.0)),
                    skip_rs=bool(np.all(rscale == 1.0)))
    in_maps = [_prep_core(weight, bias, rscale, x, core) for core in range(8)]
    args = [np.concatenate([in_maps[c][n] for c in range(8)], axis=0)
            for n in r["in_names"]]
    args += [np.zeros((8 * z.shape[0], *z.shape[1:]), z.dtype) for z in r["zero_outs"]]
    outs = r["fn"](*args)
    oi = r["out_names"].index("outp")
    o = np.asarray(outs[oi]).reshape(8, *r["out_avals"][oi].shape)
    return _unshard([o[c] for c in range(8)])


# ------------------------------------------------------------- numpy fallback

def _numpy_kernel(weight, bias, rscale, x, it, bs):
    w = weight.astype(np.float32)
    lead = (27,)
    def blockify_param(p):
        ld = p.shape[:-3]
        M, N, K = p.shape[-3:]
        y = p.reshape(*ld, M // bs, bs, N // bs, bs, K // bs, bs)
        nl = len(ld)
        perm = tuple(range(nl)) + (nl, nl + 2, nl + 4, nl + 1, nl + 3, nl + 5)
        return np.transpose(y, perm)
    wq = blockify_param(w)
    bq = blockify_param(bias.astype(np.float32))
    rsq = blockify_param(rscale.astype(np.float32))
    xp = np.pad(x.astype(np.float32), ((0, 0), (1, 1), (1, 1), (1, 1)))
    swv = np.lib.stride_tricks.sliding_window_view(xp, (bs + 2, bs + 2, bs + 2), axis=(1, 2, 3))
    acts = np.ascontiguousarray(swv[:, ::bs, ::bs, ::bs])
    inter = slice(1, bs + 1)
    for _ in range(it):
        mup = acts.mean(axis=(-3, -2, -1), keepdims=True)
        var = acts.var(axis=(-3, -2, -1), keepdims=True)
        istd = 1.0 / np.sqrt(var + EPS)
        normed = (acts - mup) * istd
        acc = np.broadcast_to(bq[None], (x.shape[0],) + bq.shape).copy()
        o = 0
        for di in range(3):
            for dj in range(3):
                for dk in range(3):
                    acc += wq[o][None] * normed[..., di:di + bs, dj:dj + bs, dk:dk + bs]
                    o += 1
        silu = acc / (1.0 + np.exp(-acc))
        acts[..., inter, inter, inter] += rsq[None] * silu
    out = acts[..., inter, inter, inter]
    B, M, N, K = x.shape
    out = out.transpose(0, 1, 4, 2, 5, 3, 6).reshape(B, M, N, K)
    return out.astype(np.float32)


# ------------------------------------------------------------- entry point

def kernel(weight, bias, residual_scale, x, inner_iterations, block_size):
    weight = np.asarray(weight, np.float32)
    bias = np.asarray(bias, np.float32)
    residual_scale = np.asarray(residual_scale, np.float32)
    x = np.asarray(x, np.float32)
    it = int(inner_iterations)
    bs = int(block_size)
    if it == 8 and bs == 8 and x.shape == (4, 128, 128, 128):
        try:
            return _run_device(weight, bias, residual_scale, x).astype(np.float32)
        except Exception:
            import traceback
            traceback.print_exc()
    return _numpy_kernel(weight, bias, residual_scale, x, it, bs)
